# revision 5
# baseline (speedup 1.0000x reference)
"""KAN layer kernel for 8 Trainium2 NeuronCores.

Math (reference):
    basis[b,f] = sum_h silu(x[b,f]*w1[f%K,h] + b1[f%K,h]) * w2[f%K,h] + b2[f%K]
    out[b,o]   = sum_f basis[b,f] * Wsum[o,f],   Wsum = W.sum(-1)   # [O,F]

Sharding: features split 8 ways (each core holds ~2048 of the 16384
features and produces a partial out[64,1024]; host sums the partials).

Per-core device program (memory-bound on streaming its ~21 MB W slice):
  - W is cast to fp16 on the host (tolerance is 2e-2; fp16 keeps us ~5e-4)
    and laid out [k, tile, partition, o] so the k-reduction folds into the
    matmul contraction: no accum-DMA chains, no on-chip k-sum. The slice
    streams as ~1 MB HWDGE DMAs at near HBM rate on the SP queue, which
    owns the SDMA engines from the first kernel instruction. All W tiles
    are SBUF-resident (fp16 fits), so the stream never stalls on compute.
  - Features are permuted so each SBUF partition only holds features of a
    single f%K residue class. The silu affine params are then per-partition
    constants valid across every tile, so the basis needs just 16 wide ACT
    instructions (one per hidden unit, N=1088) + 16 wide DVE accumulates,
    instead of 256+256 narrow ones (ACT costs (N+352)/1.2 ns per instr --
    narrow instructions are overhead-dominated).
  - Slot grid is 17 deep per partition: 16 full [128 x 64b] basis tiles plus
    one 32-partition spill tile absorbing the residue-class remainders
    (class sizes aren't multiples of 16).
  - 2 PSUM banks accumulate out[64,1024] across all 85 contraction tiles;
    the final chunks taper to 512 KB and the spill matmuls run before the
    last chunk so the post-stream tail is just 4 matmuls + output copy.
"""
import numpy as np

B, I, O, K, H = 64, 16384, 1024, 5, 16
NCORES = 8
P = 128                   # SBUF partitions
NT = 16                   # full tiles (main slot grid depth)
T = NT + 1                # slots per partition incl. spill slot
M = 32                    # spill-tile partition count
TB = T * B                # basis free dim: 17*64 = 1088
C4, N4 = 4, 18            # 18 chunks of 4 tiles (1 MB each) ...
C2, N2 = 2, 4             # ... then 4 taper chunks of 2 tiles (512 KB)
PR = 3 * H + 1            # per-partition silu params: w1,b1,w2 + b2

TRACE = False             # test.py sets True to capture an NTFF profile
LAST_RESULT = None


def _plan_core(feats_by_class):
    """Assign one core's features to the (partition, slot) grid.

    Returns (cls_of_p[P], F17[P, T]) with F17 holding feature ids, -1 = pad.
    Every partition holds features of exactly one f%K class; spill slots
    (slot NT) only on partitions < M.
    """
    ks = [len(f) for f in feats_by_class]
    n = [-(-k // T) for k in ks]              # ceil(k/17) partitions minimum
    spare = P - sum(n)
    assert spare >= 0, (ks, n)
    for _ in range(spare):                     # kill the biggest spills first
        spills = [max(0, ks[c] - NT * n[c]) for c in range(K)]
        c = int(np.argmax(spills))
        n[c] += 1
    units = []                                 # (class, main[<=16], spill|-1)
    for c in range(K):
        fs = list(feats_by_class[c])
        main, sp = fs[: NT * n[c]], fs[NT * n[c]:]
        assert len(sp) <= n[c]
        for i in range(n[c]):
            units.append((c, main[NT * i: NT * (i + 1)],
                          sp[i] if i < len(sp) else -1))
    units.sort(key=lambda u: u[2] < 0)         # spill-carrying partitions first
    assert len(units) <= P
    n_spill = sum(1 for u in units if u[2] >= 0)
    assert n_spill <= M, n_spill
    units += [(0, [], -1)] * (P - len(units))
    cls_of_p = np.array([u[0] for u in units], dtype=np.int64)
    F17 = np.full((P, T), -1, dtype=np.int64)
    for p, (c, main, sp) in enumerate(units):
        F17[p, : len(main)] = main
        F17[p, NT] = sp
    return cls_of_p, F17


def _build():
    from contextlib import ExitStack
    from concourse import bacc, mybir, tile

    f32, f16 = mybir.dt.float32, mybir.dt.float16
    mult, add = mybir.AluOpType.mult, mybir.AluOpType.add
    nc = bacc.Bacc("TRN2", target_bir_lowering=False, debug=False,
                   num_devices=NCORES)
    Wm4 = nc.declare_dram_parameter("Wm4", [N4, P, C4 * O], f16, isOutput=False)
    Wm2 = nc.declare_dram_parameter("Wm2", [N2, P, C2 * O], f16, isOutput=False)
    Wp = nc.declare_dram_parameter("Wp", [M, K * O], f16, isOutput=False)
    xd = nc.declare_dram_parameter("xd", [P, TB], f16, isOutput=False)
    prd = nc.declare_dram_parameter("pr", [P, PR], f32, isOutput=False)
    out = nc.declare_dram_parameter("out", [B, O], f32, isOutput=True)

    with tile.TileContext(nc) as tc, ExitStack() as ctx:
        const = ctx.enter_context(tc.tile_pool(name="const", bufs=1))
        wp4 = ctx.enter_context(tc.tile_pool(name="w4", bufs=N4))
        wp2 = ctx.enter_context(tc.tile_pool(name="w2", bufs=N2))
        wsp = ctx.enter_context(tc.tile_pool(name="wsp", bufs=1))
        spool = ctx.enter_context(tc.tile_pool(name="silu", bufs=4))
        apool = ctx.enter_context(tc.tile_pool(name="acc", bufs=1))
        opool = ctx.enter_context(tc.tile_pool(name="out", bufs=1))
        psum = ctx.enter_context(tc.tile_pool(name="psum", bufs=1, space="PSUM"))

        # The 21 MB W stream owns the SP HWDGE queue from instruction 0;
        # everything the basis needs rides the ACT HWDGE queue in parallel.
        wtiles = []
        for c in range(N4):
            wt = wp4.tile([P, C4 * O], f16, tag="w4", name=f"w4_{c}")
            nc.sync.dma_start(wt[:, :], Wm4[c, :, :])
            wtiles.append((wt, C4))
        for c in range(N2):
            wt = wp2.tile([P, C2 * O], f16, tag="w2", name=f"w2_{c}")
            nc.sync.dma_start(wt[:, :], Wm2[c, :, :])
            wtiles.append((wt, C2))

        xt = const.tile([P, TB], f16)
        nc.scalar.dma_start(xt[:, :], xd[:, :])
        prt = const.tile([P, PR], f32)
        nc.scalar.dma_start(prt[:, :], prd[:, :])
        wpt = wsp.tile([M, K * O], f16)
        nc.scalar.dma_start(wpt[:, :], Wp[:, :])

        # ---- basis: acc[p, t*B+b] = sum_h silu(x*w1+b1)*w2 + b2, fp16 ----
        acc = apool.tile([P, TB], f16)
        for h in range(H):
            st = spool.tile([P, TB], f16, tag="st")
            nc.scalar.activation(
                st[:, :], xt[:, :], mybir.ActivationFunctionType.Silu,
                bias=prt[:, H + h:H + h + 1], scale=prt[:, h:h + 1])
            if h == 0:
                nc.vector.tensor_scalar(
                    acc[:, :], st[:, :],
                    prt[:, 2 * H:2 * H + 1], prt[:, 3 * H:3 * H + 1],
                    op0=mult, op1=add)
            else:
                nc.vector.scalar_tensor_tensor(
                    acc[:, :], st[:, :], prt[:, 2 * H + h:2 * H + h + 1],
                    acc[:, :], op0=mult, op1=add)

        # ---- matmuls: out[b,o] += acc_tile.T @ W_tile over 85 tiles ----
        ps0 = psum.tile([B, 512], f32, tag="ps0")
        ps1 = psum.tile([B, 512], f32, tag="ps1")
        nmm = K * NT + K
        n = 0

        def mm(lhsT, r0, r1):
            nonlocal n
            nc.tensor.matmul(ps0[:, :], lhsT, r0,
                             start=(n == 0), stop=(n == nmm - 1))
            nc.tensor.matmul(ps1[:, :], lhsT, r1,
                             start=(n == 0), stop=(n == nmm - 1))
            n += 1

        tau = 0
        for ci, (wt, sz) in enumerate(wtiles):
            if ci == len(wtiles) - 1:          # spill runs before last chunk
                for k in range(K):
                    mm(acc[0:M, NT * B:TB],
                       wpt[0:M, k * O:k * O + 512],
                       wpt[0:M, k * O + 512:(k + 1) * O])
            for g in range(sz):
                t = tau % NT
                mm(acc[:, t * B:(t + 1) * B],
                   wt[:, g * O:g * O + 512],
                   wt[:, g * O + 512:(g + 1) * O])
                tau += 1
        assert tau == K * NT and n == nmm

        out_sb = opool.tile([B, O], f32)
        nc.vector.tensor_copy(out_sb[:, 0:512], ps0[:, :])
        nc.scalar.copy(out_sb[:, 512:O], ps1[:, :])
        nc.sync.dma_start(out[:, :], out_sb[:, :])
    nc.compile()
    return nc


def kernel(x, w1, b1, w2, b2, W):
    global LAST_RESULT
    from concourse.bass_utils import run_bass_kernel_spmd

    x = np.asarray(x, dtype=np.float32)
    W = np.asarray(W, dtype=np.float32)
    w1 = np.asarray(w1, dtype=np.float32)
    b1 = np.asarray(b1, dtype=np.float32)
    w2 = np.asarray(w2, dtype=np.float32)
    b2 = np.asarray(b2, dtype=np.float32)

    # ---- host prep: W -> fp16 [f, k, o] with a zero row for pad slots ----
    Wt = np.zeros((I + 1, K, O), dtype=np.float16)
    Wt[:I] = W.transpose(1, 2, 0)
    xp = np.concatenate([x, np.zeros((B, 1), np.float32)], axis=1)

    in_maps = []
    seen = []
    for j in range(NCORES):
        feats = [np.arange(c, I, K)[j::NCORES] for c in range(K)]
        cls_of_p, F17 = _plan_core(feats)
        seen.append(F17[F17 >= 0].ravel())

        Fx = np.where(F17 < 0, I, F17)                 # pad -> zero col/row
        x_sb = np.ascontiguousarray(
            xp[:, Fx].transpose(1, 2, 0).reshape(P, TB).astype(np.float16))
        pr = np.ascontiguousarray(np.concatenate(
            [w1[cls_of_p], b1[cls_of_p], w2[cls_of_p], b2[cls_of_p][:, None]],
            axis=1, dtype=np.float32))

        A = Wt[Fx[:, :NT].T]                            # [t, p, k, o] fp16
        A = A.transpose(2, 0, 1, 3).reshape(K * NT, P, O)   # [tau, p, o]
        n4 = N4 * C4
        Wm4 = np.ascontiguousarray(
            A[:n4].reshape(N4, C4, P, O).transpose(0, 2, 1, 3)
            .reshape(N4, P, C4 * O))
        Wm2 = np.ascontiguousarray(
            A[n4:].reshape(N2, C2, P, O).transpose(0, 2, 1, 3)
            .reshape(N2, P, C2 * O))
        Wp_img = np.ascontiguousarray(Wt[Fx[:M, NT]].reshape(M, K * O))
        in_maps.append({"Wm4": Wm4, "Wm2": Wm2, "Wp": Wp_img,
                        "xd": x_sb, "pr": pr})

    allf = np.sort(np.concatenate(seen))
    assert allf.shape == (I,) and np.array_equal(allf, np.arange(I))

    nc = _build()
    res = run_bass_kernel_spmd(nc, in_maps, list(range(NCORES)), trace=TRACE)
    LAST_RESULT = res
    out = np.zeros((B, O), dtype=np.float32)
    for c in range(NCORES):
        out += res.results[c]["out"]
    return out
